# revision 24
# baseline (speedup 1.0000x reference)
"""Causal cross-attention kernel for 8 TRN2 NeuronCores.

Sharding: data-parallel over batch (B=2) x tensor-parallel over head
groups (16 heads -> 4 groups of 4). Core c handles batch c//4, heads
[4*(c%4), 4*(c%4)+4). Each core computes its partial output projection
(w_out rows for its heads); the host sums the 4 partials per batch
(the "all-reduce"), adds b_out, and fixes the fully-masked row 0.

v2 design notes (vs the first working kernel):
  - s-chunk pipeline: project chunk sc's Q/K/V while running attention
    for chunk sc-1 and the output projection for chunk sc-2, woven into
    one emission stream so the in-order PE queue always has non-ACT-
    gated matmuls mixed between the exp-dependent AV matmuls. This
    keeps PE activity high enough for HAM to hold K=8/8 (the v1 kernel
    sat at 1.2 GHz for 225us because attention alone left the PE ~60%
    busy, below the un-throttle threshold).
  - Scores for a head PAIR run as two concurrent K=64 row-tiled
    matmuls (partitions 0-63 / 64-127) into the two banks of one
    [128, 2, 512] PSUM slab; one joint exp covers both heads.
  - Diagonal z-blocks compute only the live columns (fully-masked
    columns are skipped in the score MM, exp, and AV MM); the causal
    boundary is one constant [128,128] triangle mask tile, added on DVE.
  - P and V are bf16 (the AV matmul is linear in both; bf16 keeps it
    full-rate and enables fast weight load). Q/K/scores stay f32r.
  - Softmax denominator comes free as row 64 of the AV accumulation
    (ones column in v1); its reciprocal uses the single-pass
    reciprocal_approx_fast (~18 bits) instead of the 6.5 cyc/elem
    exact reciprocal (which cost 3.35us per [1,512] row in v1).
  - PSUM budget (8 banks): score slabs 2x2 + pav 2 + misc(proj/out/pb) 2.
"""

import numpy as np
import concourse.bass as bass
import concourse.mybir as mybir
import concourse.tile as tile
from concourse.bass_utils import run_bass_kernel_spmd

B, S, F, H = 2, 2048, 1024, 16
NCORES = 8
HG = 4          # head groups (tensor-parallel degree per batch)
HPC = H // HG   # heads per core = 4
D = F // H      # head dim = 64
CW = HPC * D    # per-core projection width = 256
MASK_VAL = 1.0e12
SC = 512        # s-chunk
NSC = S // SC   # 4
NZC = S // 128  # 16 z-blocks

f32 = mybir.dt.float32
f32r = mybir.dt.float32r
bf16 = mybir.dt.bfloat16

# Walrus encodes at most 1 sync wait on most TRN2 instructions; Tile can
# attach several. Redistribute excess waits onto preceding same-engine NOPs.


def _split_excess_waits(nc):
    for fn in nc.m.functions:
        for bb in fn.blocks:
            insts = list(bb.instructions)
            out = []
            changed = False
            for inst in insts:
                si = inst.sync_info
                waits = list(si.on_wait) if si is not None else []
                if len(waits) > 1:
                    changed = True
                    inst.sync_info = mybir.SyncInfo(
                        on_update=list(si.on_update), on_wait=waits[-1:]
                    )
                    for idx, w in enumerate(waits[:-1]):
                        nop = mybir.InstNoOp(name=f"{inst.name}-wsplit{idx}")
                        nop.engine = inst.engine
                        nop.sync_info = mybir.SyncInfo(on_update=[], on_wait=[w])
                        out.append(nop)
                out.append(inst)
            if changed:
                bb.instructions = out


def _round_f32r(x):
    u = np.ascontiguousarray(x, dtype=np.float32).view(np.uint32)
    u = ((u.astype(np.uint64) + 0x1000) & 0xFFFFE000).astype(np.uint32)
    return u.view(np.float32)


def _weave(primary, early, late):
    """Emit thunks: `early` distributed over the first third of `primary`
    groups (projection fills that later groups depend on), `late` over the
    second half (deferred tails that must not head-of-line-block the
    attention stream)."""
    np_ = len(primary)
    ne, nl = len(early), len(late)
    n1 = max(1, np_ // 3)
    n2 = np_ - np_ // 2
    ei = li = 0
    for i, g in enumerate(primary):
        g()
        ewant = min(ne, ((i + 1) * ne) // n1) if n1 else ne
        while ei < ewant:
            early[ei]()
            ei += 1
        lwant = max(0, ((i + 1 - (np_ - n2)) * nl) // n2) if n2 else nl
        while li < min(nl, lwant):
            late[li]()
            li += 1
    while ei < ne:
        early[ei]()
        ei += 1
    while li < nl:
        late[li]()
        li += 1


def _build():
    nc = bass.Bass()
    xf_d = nc.declare_dram_parameter("xf", [F, S], f32r, isOutput=False)
    xt_d = nc.declare_dram_parameter("xt", [F, S], f32r, isOutput=False)
    wq_d = nc.declare_dram_parameter("wq", [F, CW], f32r, isOutput=False)
    wk_d = nc.declare_dram_parameter("wk", [F, CW], f32r, isOutput=False)
    wv_d = nc.declare_dram_parameter("wv", [F, CW], f32r, isOutput=False)
    wo_d = nc.declare_dram_parameter("wo", [CW, F], f32r, isOutput=False)
    bq_d = nc.declare_dram_parameter("bq", [CW, 1], f32, isOutput=False)
    bk_d = nc.declare_dram_parameter("bk", [CW, 1], f32, isOutput=False)
    bv_d = nc.declare_dram_parameter("bv", [1, CW], f32r, isOutput=False)
    mskA_d = nc.declare_dram_parameter("mskA", [128, 128], f32r, isOutput=False)
    idn_d = nc.declare_dram_parameter("idn", [128, 128], f32r, isOutput=False)
    ind4_d = nc.declare_dram_parameter("ind4", [64, 2 * 64], f32r, isOutput=False)
    out_d = nc.declare_dram_parameter("out", [S, F], f32, isOutput=True)

    with tile.TileContext(nc) as tc:
        with (
            tc.tile_pool(name="const", bufs=1) as cpool,
            tc.tile_pool(name="xf", bufs=2) as xfpool,
            tc.tile_pool(name="xt", bufs=2) as xtpool,
            tc.tile_pool(name="p2", bufs=3) as p2pool,
            tc.tile_pool(name="aou", bufs=6) as aopool,
            tc.tile_pool(name="rc", bufs=2) as rcpool,
            tc.tile_pool(name="ost", bufs=2) as opool,
            tc.tile_pool(name="ps2", bufs=2, space="PSUM") as ps2pool,
            tc.tile_pool(name="pav", bufs=2, space="PSUM") as pavpool,
            tc.tile_pool(name="misc", bufs=2, space="PSUM") as mpool,
        ):
            # ---- persistent tiles ----
            wq = cpool.tile([128, 8, CW], f32r)
            wk = cpool.tile([128, 8, CW], f32r)
            wv = cpool.tile([128, 8, CW], f32r)
            wo = cpool.tile([128, 2, F], f32r)
            bq = cpool.tile([128, 2, 1], f32)
            bk = cpool.tile([128, 2, 1], f32)
            bv = cpool.tile([1, CW], f32r)
            mskA = cpool.tile([128, 128], f32r)
            idn = cpool.tile([128, 128], f32r)
            ind4 = cpool.tile([64, 2 * 64], f32r)
            denT0 = cpool.tile([128, SC], f32)
            denT1 = cpool.tile([128, SC], f32)
            ones_f = cpool.tile([128, 128], f32)
            ones_r = cpool.tile([1, 128], f32r)
            qT = cpool.tile([128, 2, S], f32r)
            kT = cpool.tile([128, 2, S], f32r)
            v1 = cpool.tile([128, NZC, HPC, D + 1], bf16)
            aoT = cpool.tile([128, 2, S], f32r)

            # wq first so Q(0) matmuls can start as soon as possible; the
            # x chunks are issued by dma_chunk(0) right after wq below.
            # All input loads go on the (otherwise idle) gpsimd queue: the
            # sync-queue DGE dispatch is ~600ns/DMA and serializing ~30 input
            # DMAs there kept the PE idle for the first 30us.
            for k in range(8):
                nc.gpsimd.dma_start(
                    out=wq[:, k, :], in_=wq_d[128 * k : 128 * (k + 1), :]
                )
            def small_consts():
                nc.sync.dma_start(
                    out=bq[:], in_=bq_d[:].rearrange("(m p) c -> p m c", p=128)
                )
                nc.sync.dma_start(
                    out=bk[:], in_=bk_d[:].rearrange("(m p) c -> p m c", p=128)
                )
                nc.sync.dma_start(out=bv[:], in_=bv_d[:])
                nc.sync.dma_start(out=mskA[:], in_=mskA_d[:])
                nc.sync.dma_start(out=idn[:], in_=idn_d[:])
                nc.sync.dma_start(out=ind4[:], in_=ind4_d[:])
            def wkv_dmas():
                for k in range(8):
                    nc.gpsimd.dma_start(
                        out=wk[:, k, :], in_=wk_d[128 * k : 128 * (k + 1), :]
                    )
                    nc.gpsimd.dma_start(
                        out=wv[:, k, :], in_=wv_d[128 * k : 128 * (k + 1), :]
                    )

            def late_consts():
                nc.gpsimd.dma_start(
                    out=wo[:], in_=wo_d[:].rearrange("(m p) c -> p m c", p=128)
                )

            nc.vector.memset(ones_f[:], 1.0)
            nc.vector.tensor_copy(ones_r[:], ones_f[0:1, :])
            nc.vector.memset(denT0[:], 1.0)
            nc.vector.memset(denT1[:], 1.0)
            # ones column of v' (col D of each [128, D+1] block)
            nc.vector.tensor_copy(
                v1[:, :, :, D : D + 1],
                ones_f[:, :64].rearrange("p (a b c) -> p a b c", a=NZC, b=HPC),
            )

            xtiles = {}

            def dma_chunk(sc):
                def go():
                    s0 = sc * SC
                    xf = xfpool.tile([128, 8, SC], f32r, tag="xf")
                    xt = xtpool.tile([128, 8, SC], f32r, tag="xt")
                    for k in range(8):
                        nc.sync.dma_start(
                            out=xf[:, k, :],
                            in_=xf_d[128 * k : 128 * (k + 1), s0 : s0 + SC],
                        )
                        nc.gpsimd.dma_start(
                            out=xt[:, k, :],
                            in_=xt_d[128 * k : 128 * (k + 1), s0 : s0 + SC],
                        )
                    xtiles[sc] = (xf, xt)

                return go

            # ---- projection thunks for chunk sc ----
            def proj_thunks(sc):
                s0 = sc * SC
                thunks = []

                def qk(which, m):
                    def go():
                        xf, xt = xtiles[sc]
                        w, x, bias, dst = {
                            "q": (wq, xf, bq, qT),
                            "k": (wk, xt, bk, kT),
                        }[which]
                        p = mpool.tile([128, SC], f32, tag="misc")
                        for k in range(8):
                            nc.tensor.matmul(
                                p[:],
                                w[:, k, m * 128 : (m + 1) * 128],
                                x[:, k, :],
                                start=(k == 0),
                                stop=(k == 7),
                            )
                        if which == "q":
                            nc.vector.tensor_scalar_add(
                                dst[:, m, s0 : s0 + SC], p[:], bias[:, m, :]
                            )
                        else:
                            nc.scalar.activation(
                                dst[:, m, s0 : s0 + SC],
                                p[:],
                                mybir.ActivationFunctionType.Identity,
                                bias=bias[:, m, :],
                            )

                    return go

                def vproj(zz):
                    def go():
                        xf, xt = xtiles[sc]
                        zc = sc * 4 + zz
                        p = mpool.tile([128, SC], f32, tag="misc")
                        for k in range(8):
                            nc.tensor.matmul(
                                p[:, :CW],
                                xt[:, k, zz * 128 : (zz + 1) * 128],
                                wv[:, k, :],
                                start=(k == 0),
                                stop=False,
                            )
                        nc.tensor.matmul(
                            p[:, :CW], ones_r[:], bv[:], start=False, stop=True
                        )
                        nc.scalar.copy(v1[:, zc, :, 0:D], p[:, :CW])

                    return go

                for m in range(2):
                    thunks.append(qk("q", m))
                    thunks.append(qk("k", m))
                for zz in range(4):
                    thunks.append(vproj(zz))
                return thunks

            # ---- attention groups for chunk sc ----
            # tails[sc] collects deferred thunks: broadcast+normalize per
            # head, then the output projection for the chunk.
            tails = {sc: [] for sc in range(NSC)}

            def attn_groups(sc):
                s0 = sc * SC
                groups = []
                aos = {}
                for m in range(2):  # head pair (heads 2m, 2m+1)
                    nz = 4 * sc + 4
                    pavs = {}

                    def start_pair(m=m):
                        def go():
                            pavs[0] = pavpool.tile(
                                [D + 1, SC], f32, tag="pav", name="pav0"
                            )
                            pavs[1] = pavpool.tile(
                                [D + 1, SC], f32, tag="pav", name="pav1"
                            )

                        return go

                    def zgroup(zc, m=m, nz=nz):
                        def go():
                            diag = zc >= 4 * sc
                            j0 = 128 * (zc - 4 * sc) if diag else 0
                            ps = ps2pool.tile([128, 2, SC], f32, tag="ps2")
                            for hh in range(2):
                                po = 64 * hh
                                nc.tensor.matmul(
                                    ps[:, hh, j0:],
                                    kT[po : po + D, m, 128 * zc : 128 * (zc + 1)],
                                    qT[po : po + D, m, s0 + j0 : s0 + SC],
                                    start=True,
                                    stop=not diag,
                                )
                            if diag:
                                for hh in range(2):
                                    nc.tensor.matmul(
                                        ps[:, hh, j0 : j0 + 128],
                                        mskA[:],
                                        idn[:],
                                        start=False,
                                        stop=True,
                                    )
                            p2 = p2pool.tile([128, 2, SC], bf16, tag="p2")
                            nc.scalar.activation(
                                p2[:, :, j0:],
                                ps[:, :, j0:],
                                mybir.ActivationFunctionType.Exp,
                            )
                            for hh in range(2):
                                h = 2 * m + hh
                                nc.tensor.matmul(
                                    pavs[hh][:, j0:],
                                    v1[:, zc, h, :],
                                    p2[:, hh, j0:],
                                    start=(zc == 0),
                                    stop=(zc == nz - 1),
                                )

                        return go

                    def end_pair(m=m):
                        def go():
                            denT = denT0 if m == 0 else denT1
                            # cast h first (frees the pav bank for the next
                            # pair ASAP), its den row right after, then the
                            # reciprocal once both rows are in.
                            for hh in range(2):
                                g = 2 * m + hh
                                ao = aopool.tile([D + 1, SC], f32r, tag="aoTu")
                                nc.vector.tensor_copy(ao[:], pavs[hh][:])
                                nc.vector.tensor_copy(
                                    denT[32 * hh : 32 * hh + 1, :],
                                    ao[D : D + 1, :],
                                )
                                aos[g] = ao
                            rci = rcpool.tile([D, SC], f32, tag="rci")
                            nc.vector.reciprocal(rci[:], denT[0:D, :])
                            recT = rcpool.tile([D, SC], f32r, tag="recT", bufs=4)
                            nc.vector.tensor_scalar_min(recT[:], rci[:], 1.0e30)
                            for hh in range(2):
                                tails[sc].append(
                                    norm_thunk(m, hh, aos[2 * m + hh], recT)
                                )

                        return go

                    def norm_thunk(m, hh, ao, recT):
                        def go():
                            po = 64 * hh
                            pbd = mpool.tile([128, SC], f32, tag="misc")
                            nc.tensor.matmul(
                                pbd[0:D, :],
                                ind4[:, hh * D : (hh + 1) * D],
                                recT[:],
                                start=True,
                                stop=True,
                            )
                            nc.vector.tensor_tensor(
                                out=aoT[po : po + D, m, s0 : s0 + SC],
                                in0=ao[0:D, :],
                                in1=pbd[0:D, :],
                                op=mybir.AluOpType.mult,
                            )

                        return go

                    groups.append(start_pair())
                    gs = [zgroup(zc) for zc in range(nz)]
                    if m == 1 and sc >= 1:
                        # flush pair0's deferred normalize (bcast+mult) into
                        # pair1's attention stream, well after pair0's recip
                        # chain has drained on DVE.
                        def flush():
                            todo = tails[sc][:]
                            tails[sc].clear()
                            for t in todo:
                                t()

                        gs = gs[:7] + [flush] + gs[7:]
                    groups.extend(gs)
                    groups.append(end_pair())
                return groups

            # ---- output projection thunks for chunk sc ----
            def p3_thunks(sc):
                thunks = []
                for sb in range(4):
                    def go(sb=sb):
                        s0b = sc * SC + sb * 128
                        ost = opool.tile([128, 2, SC], f32, tag="ost")
                        po0 = mpool.tile([128, SC], f32, tag="misc")
                        po1 = mpool.tile([128, SC], f32, tag="misc")
                        for m in range(2):
                            nc.tensor.matmul(
                                po0[:],
                                aoT[:, m, s0b : s0b + 128],
                                wo[:, m, 0:SC],
                                start=(m == 0),
                                stop=(m == 1),
                            )
                            nc.tensor.matmul(
                                po1[:],
                                aoT[:, m, s0b : s0b + 128],
                                wo[:, m, SC : 2 * SC],
                                start=(m == 0),
                                stop=(m == 1),
                            )
                        nc.vector.tensor_copy(ost[:, 0, :], po0[:])
                        nc.sync.dma_start(
                            out=out_d[s0b : s0b + 128, 0:SC], in_=ost[:, 0, :]
                        )
                        nc.scalar.copy(ost[:, 1, :], po1[:])
                        nc.sync.dma_start(
                            out=out_d[s0b : s0b + 128, SC:], in_=ost[:, 1, :]
                        )

                    thunks.append(go)
                return thunks

            # ---- emission schedule ----
            # proj_thunks order: [q(m0), k(m0), q(m1), k(m1), v0..v3].
            # For the last chunk, K and V weave into attn(3) itself so the
            # tail attention stretch (which has no other projection work
            # left) keeps enough non-exp-gated PE work to hold HAM warm.
            dma_chunk(0)()
            small_consts()
            wkv_dmas()
            p0 = proj_thunks(0)
            for t in p0[:3]:
                t()
            dma_chunk(1)()
            late_consts()
            for t in p0[3:]:
                t()
            carry = []
            for sc in range(1, NSC + 1):
                early, late = [], []
                early.extend(carry)
                carry = []
                if sc < NSC:
                    if sc + 1 < NSC:
                        early.append(dma_chunk(sc + 1))
                    pt = proj_thunks(sc)
                    if sc == NSC - 1:
                        early.extend([pt[0], pt[2]])
                        carry = [pt[1], pt[3]] + pt[4:]
                    else:
                        early.extend(pt)
                if sc >= 2:
                    late.extend(tails[sc - 2])
                    late.extend(p3_thunks(sc - 2))
                _weave(attn_groups(sc - 1), early, late)
            for t in tails[NSC - 1]:
                t()
            for t in p3_thunks(NSC - 1):
                t()

    _split_excess_waits(nc)
    return nc


_CACHE = {}


def _get_nc():
    if "nc" not in _CACHE:
        _CACHE["nc"] = _build()
    return _CACHE["nc"]


def _ensure_ntff_hook():
    """The agent image's antenv lacks axon_hooks, so run_bass_kernel_spmd's
    trace path can't import it. Synthesize the module and install the
    ctypes NTFF hook from trn_agent_boot (same thing boot() would do)."""
    import sys
    import types

    if "antenv.axon_hooks" not in sys.modules:
        mod = types.ModuleType("antenv.axon_hooks")
        holder = [None]
        mod.set_axon_ntff_profile_hook = lambda h: holder.__setitem__(0, h)
        mod.get_axon_ntff_profile_hook = lambda: holder[0]
        sys.modules["antenv.axon_hooks"] = mod
        import antenv

        antenv.axon_hooks = mod
    import antenv.axon_hooks as ah

    if ah.get_axon_ntff_profile_hook() is None:
        try:
            from trn_agent_boot.trn_boot import _ntff_profile_via_ctypes

            ah.set_axon_ntff_profile_hook(
                _ntff_profile_via_ctypes("/opt/axon/libaxon_pjrt.so")
            )
        except Exception:
            pass


def _host_consts():
    # mskA[p, i] = -MASK_VAL if i >= p else 0, so that (mskA.T @ I)[i, j]
    # = mskA[j, i] = -MASK_VAL * (i >= j)  (the causal triangle).
    p = np.arange(128)[:, None]
    i = np.arange(128)[None, :]
    mskA = np.where(i >= p, -np.float32(MASK_VAL), np.float32(0.0)).astype(
        np.float32
    )
    idn = np.eye(128, dtype=np.float32)
    ind4 = np.zeros((64, 2 * D), np.float32)
    for hh in range(2):
        ind4[32 * hh, hh * D : (hh + 1) * D] = 1.0
    return _round_f32r(mskA), _round_f32r(idn), _round_f32r(ind4)


def kernel(attend_from, attend_to, w_q, b_q, w_kv, b_kv, w_out, b_out, _trace=False):
    attend_from = np.asarray(attend_from, dtype=np.float32)
    attend_to = np.asarray(attend_to, dtype=np.float32)
    w_q = np.asarray(w_q, dtype=np.float32)
    b_q = np.asarray(b_q, dtype=np.float32)
    w_kv = np.asarray(w_kv, dtype=np.float32)
    b_kv = np.asarray(b_kv, dtype=np.float32)
    w_out = np.asarray(w_out, dtype=np.float32)
    b_out = np.asarray(b_out, dtype=np.float32)

    mskA, idn, ind4 = _host_consts()
    xT = [_round_f32r(attend_from[b].T) for b in range(B)]
    xTt = [_round_f32r(attend_to[b].T) for b in range(B)]

    in_maps = []
    for c in range(NCORES):
        b, hg = divmod(c, HG)
        cols = slice(hg * CW, (hg + 1) * CW)
        in_maps.append(
            {
                "xf": xT[b],
                "xt": xTt[b],
                "wq": _round_f32r(w_q[:, cols]),
                "wk": _round_f32r(w_kv[:, cols]),
                "wv": _round_f32r(w_kv[:, F:][:, cols]),
                "wo": _round_f32r(w_out[cols, :]),
                "bq": np.ascontiguousarray(b_q[cols].reshape(CW, 1)),
                "bk": np.ascontiguousarray(b_kv[cols].reshape(CW, 1)),
                "bv": _round_f32r(b_kv[F:][cols].reshape(1, CW)),
                "mskA": mskA,
                "idn": idn,
                "ind4": ind4,
                "out": np.zeros((S, F), np.float32),
            }
        )

    nc = _get_nc()
    if _trace:
        _ensure_ntff_hook()
    res = run_bass_kernel_spmd(nc, in_maps, list(range(NCORES)), trace=_trace)

    out = np.zeros((B, S, F), np.float64)
    for c in range(NCORES):
        b = c // HG
        out[b] += res.results[c]["out"].astype(np.float64)
    out += b_out.astype(np.float64)[None, None, :]

    # Row 0 of the reference is fully masked -> softmax is exactly uniform
    # over all Z positions; compute it directly on the host.
    w_v = w_kv[:, F:].astype(np.float64)
    for b in range(B):
        val_mean = attend_to[b].astype(np.float64).mean(axis=0) @ w_v + b_kv[
            F:
        ].astype(np.float64)
        out[b, 0, :] = val_mean @ w_out.astype(np.float64) + b_out.astype(np.float64)

    if _trace:
        kernel._last_result = res
    return out.astype(np.float32)


# revision 25
# speedup vs baseline: 1.0837x; 1.0837x over previous
"""Causal cross-attention kernel for 8 TRN2 NeuronCores.

Sharding: data-parallel over batch (B=2) x tensor-parallel over head
groups (16 heads -> 4 groups of 4). Core c handles batch c//4, heads
[4*(c%4), 4*(c%4)+4). Each core computes its partial output projection
(w_out rows for its heads); the host sums the 4 partials per batch
(the "all-reduce"), adds b_out, and fixes the fully-masked row 0.

v2 design notes (vs the first working kernel):
  - s-chunk pipeline: project chunk sc's Q/K/V while running attention
    for chunk sc-1 and the output projection for chunk sc-2, woven into
    one emission stream so the in-order PE queue always has non-ACT-
    gated matmuls mixed between the exp-dependent AV matmuls. This
    keeps PE activity high enough for HAM to hold K=8/8 (the v1 kernel
    sat at 1.2 GHz for 225us because attention alone left the PE ~60%
    busy, below the un-throttle threshold).
  - Scores for a head PAIR run as two concurrent K=64 row-tiled
    matmuls (partitions 0-63 / 64-127) into the two banks of one
    [128, 2, 512] PSUM slab; one joint exp covers both heads.
  - Diagonal z-blocks compute only the live columns (fully-masked
    columns are skipped in the score MM, exp, and AV MM); the causal
    boundary is one constant [128,128] triangle mask tile, added on DVE.
  - P and V are bf16 (the AV matmul is linear in both; bf16 keeps it
    full-rate and enables fast weight load). Q/K/scores stay f32r.
  - Softmax denominator comes free as row 64 of the AV accumulation
    (ones column in v1); its reciprocal uses the single-pass
    reciprocal_approx_fast (~18 bits) instead of the 6.5 cyc/elem
    exact reciprocal (which cost 3.35us per [1,512] row in v1).
  - PSUM budget (8 banks): score slabs 2x2 + pav 2 + misc(proj/out/pb) 2.
"""

import numpy as np
import concourse.bass as bass
import concourse.mybir as mybir
import concourse.tile as tile
from concourse.bass_utils import run_bass_kernel_spmd

B, S, F, H = 2, 2048, 1024, 16
NCORES = 8
HG = 4          # head groups (tensor-parallel degree per batch)
HPC = H // HG   # heads per core = 4
D = F // H      # head dim = 64
CW = HPC * D    # per-core projection width = 256
MASK_VAL = 1.0e12
SC = 512        # s-chunk
NSC = S // SC   # 4
NZC = S // 128  # 16 z-blocks

f32 = mybir.dt.float32
f32r = mybir.dt.float32r
bf16 = mybir.dt.bfloat16

# Walrus encodes at most 1 sync wait on most TRN2 instructions; Tile can
# attach several. Redistribute excess waits onto preceding same-engine NOPs.


def _split_excess_waits(nc):
    for fn in nc.m.functions:
        for bb in fn.blocks:
            insts = list(bb.instructions)
            out = []
            changed = False
            for inst in insts:
                si = inst.sync_info
                waits = list(si.on_wait) if si is not None else []
                if len(waits) > 1:
                    changed = True
                    inst.sync_info = mybir.SyncInfo(
                        on_update=list(si.on_update), on_wait=waits[-1:]
                    )
                    for idx, w in enumerate(waits[:-1]):
                        nop = mybir.InstNoOp(name=f"{inst.name}-wsplit{idx}")
                        nop.engine = inst.engine
                        nop.sync_info = mybir.SyncInfo(on_update=[], on_wait=[w])
                        out.append(nop)
                out.append(inst)
            if changed:
                bb.instructions = out


def _round_f32r(x):
    u = np.ascontiguousarray(x, dtype=np.float32).view(np.uint32)
    u = ((u.astype(np.uint64) + 0x1000) & 0xFFFFE000).astype(np.uint32)
    return u.view(np.float32)


def _weave(primary, early, late):
    """Emit thunks: `early` distributed over the first third of `primary`
    groups (projection fills that later groups depend on), `late` over the
    second half (deferred tails that must not head-of-line-block the
    attention stream)."""
    np_ = len(primary)
    ne, nl = len(early), len(late)
    n1 = max(1, np_ // 3)
    n2 = np_ - np_ // 2
    ei = li = 0
    for i, g in enumerate(primary):
        g()
        ewant = min(ne, ((i + 1) * ne) // n1) if n1 else ne
        while ei < ewant:
            early[ei]()
            ei += 1
        lwant = max(0, ((i + 1 - (np_ - n2)) * nl) // n2) if n2 else nl
        while li < min(nl, lwant):
            late[li]()
            li += 1
    while ei < ne:
        early[ei]()
        ei += 1
    while li < nl:
        late[li]()
        li += 1


def _build():
    nc = bass.Bass()
    xf_d = nc.declare_dram_parameter("xf", [F, S], f32r, isOutput=False)
    xt_d = nc.declare_dram_parameter("xt", [F, S], f32r, isOutput=False)
    wq_d = nc.declare_dram_parameter("wq", [F, CW], f32r, isOutput=False)
    wk_d = nc.declare_dram_parameter("wk", [F, CW], f32r, isOutput=False)
    wv_d = nc.declare_dram_parameter("wv", [F, CW], f32r, isOutput=False)
    wo_d = nc.declare_dram_parameter("wo", [CW, F], f32r, isOutput=False)
    bq_d = nc.declare_dram_parameter("bq", [CW, 1], f32, isOutput=False)
    bk_d = nc.declare_dram_parameter("bk", [CW, 1], f32, isOutput=False)
    bv_d = nc.declare_dram_parameter("bv", [1, CW], f32r, isOutput=False)
    mskA_d = nc.declare_dram_parameter("mskA", [128, 128], f32r, isOutput=False)
    idn_d = nc.declare_dram_parameter("idn", [128, 128], f32r, isOutput=False)
    ind4_d = nc.declare_dram_parameter("ind4", [64, 2 * 64], f32r, isOutput=False)
    out_d = nc.declare_dram_parameter("out", [S, F], f32, isOutput=True)

    with tile.TileContext(nc) as tc:
        with (
            tc.tile_pool(name="const", bufs=1) as cpool,
            tc.tile_pool(name="xf", bufs=2) as xfpool,
            tc.tile_pool(name="xt", bufs=2) as xtpool,
            tc.tile_pool(name="p2", bufs=3) as p2pool,
            tc.tile_pool(name="aou", bufs=6) as aopool,
            tc.tile_pool(name="rc", bufs=2) as rcpool,
            tc.tile_pool(name="ost", bufs=2) as opool,
            tc.tile_pool(name="ps2", bufs=2, space="PSUM") as ps2pool,
            tc.tile_pool(name="pav", bufs=2, space="PSUM") as pavpool,
            tc.tile_pool(name="misc", bufs=2, space="PSUM") as mpool,
        ):
            # ---- persistent tiles ----
            wq = cpool.tile([128, 8, CW], f32r)
            wk = cpool.tile([128, 8, CW], f32r)
            wv = cpool.tile([128, 8, CW], f32r)
            wo = cpool.tile([128, 2, F], f32r)
            bq = cpool.tile([128, 2, 1], f32)
            bk = cpool.tile([128, 2, 1], f32)
            bv = cpool.tile([1, CW], f32r)
            mskA = cpool.tile([128, 128], f32r)
            idn = cpool.tile([128, 128], f32r)
            ind4 = cpool.tile([64, 2 * 64], f32r)
            denT0 = cpool.tile([128, SC], f32)
            denT1 = cpool.tile([128, SC], f32)
            ones_f = cpool.tile([128, 128], f32)
            ones_r = cpool.tile([1, 128], f32r)
            qT = cpool.tile([128, 2, S], f32r)
            kT = cpool.tile([128, 2, S], f32r)
            v1 = cpool.tile([128, NZC, HPC, D + 1], bf16)
            aoT = cpool.tile([128, 2, S], f32r)

            # wq first so Q(0) matmuls can start as soon as possible; the
            # x chunks are issued by dma_chunk(0) right after wq below.
            # All input loads go on the (otherwise idle) gpsimd queue: the
            # sync-queue DGE dispatch is ~600ns/DMA and serializing ~30 input
            # DMAs there kept the PE idle for the first 30us.
            for k in range(8):
                nc.gpsimd.dma_start(
                    out=wq[:, k, :], in_=wq_d[128 * k : 128 * (k + 1), :]
                )
            def small_consts():
                nc.sync.dma_start(
                    out=bq[:], in_=bq_d[:].rearrange("(m p) c -> p m c", p=128)
                )
                nc.sync.dma_start(
                    out=bk[:], in_=bk_d[:].rearrange("(m p) c -> p m c", p=128)
                )
                nc.sync.dma_start(out=bv[:], in_=bv_d[:])
                nc.sync.dma_start(out=mskA[:], in_=mskA_d[:])
                nc.sync.dma_start(out=idn[:], in_=idn_d[:])
                nc.sync.dma_start(out=ind4[:], in_=ind4_d[:])
            def wkv_dmas():
                for k in range(8):
                    nc.gpsimd.dma_start(
                        out=wk[:, k, :], in_=wk_d[128 * k : 128 * (k + 1), :]
                    )
                    nc.gpsimd.dma_start(
                        out=wv[:, k, :], in_=wv_d[128 * k : 128 * (k + 1), :]
                    )

            def late_consts():
                nc.gpsimd.dma_start(
                    out=wo[:], in_=wo_d[:].rearrange("(m p) c -> p m c", p=128)
                )

            nc.vector.memset(ones_f[:], 1.0)
            nc.vector.tensor_copy(ones_r[:], ones_f[0:1, :])
            nc.vector.memset(denT0[:], 1.0)
            nc.vector.memset(denT1[:], 1.0)
            # ones column of v' (col D of each [128, D+1] block)
            nc.vector.tensor_copy(
                v1[:, :, :, D : D + 1],
                ones_f[:, :64].rearrange("p (a b c) -> p a b c", a=NZC, b=HPC),
            )

            xtiles = {}

            def dma_chunk(sc):
                def go():
                    s0 = sc * SC
                    xf = xfpool.tile([128, 8, SC], f32r, tag="xf")
                    xt = xtpool.tile([128, 8, SC], f32r, tag="xt")
                    for k in range(8):
                        nc.sync.dma_start(
                            out=xf[:, k, :],
                            in_=xf_d[128 * k : 128 * (k + 1), s0 : s0 + SC],
                        )
                        nc.gpsimd.dma_start(
                            out=xt[:, k, :],
                            in_=xt_d[128 * k : 128 * (k + 1), s0 : s0 + SC],
                        )
                    xtiles[sc] = (xf, xt)

                return go

            # ---- projection thunks for chunk sc ----
            def proj_thunks(sc):
                s0 = sc * SC
                thunks = []

                def qk(which, m):
                    def go():
                        xf, xt = xtiles[sc]
                        w, x, bias, dst = {
                            "q": (wq, xf, bq, qT),
                            "k": (wk, xt, bk, kT),
                        }[which]
                        p = mpool.tile([128, SC], f32, tag="misc")
                        for k in range(8):
                            nc.tensor.matmul(
                                p[:],
                                w[:, k, m * 128 : (m + 1) * 128],
                                x[:, k, :],
                                start=(k == 0),
                                stop=(k == 7),
                            )
                        if which == "q":
                            nc.vector.tensor_scalar_add(
                                dst[:, m, s0 : s0 + SC], p[:], bias[:, m, :]
                            )
                        else:
                            nc.scalar.activation(
                                dst[:, m, s0 : s0 + SC],
                                p[:],
                                mybir.ActivationFunctionType.Identity,
                                bias=bias[:, m, :],
                            )

                    return go

                def vproj(zz):
                    def go():
                        xf, xt = xtiles[sc]
                        zc = sc * 4 + zz
                        p = mpool.tile([128, SC], f32, tag="misc")
                        for k in range(8):
                            nc.tensor.matmul(
                                p[:, :CW],
                                xt[:, k, zz * 128 : (zz + 1) * 128],
                                wv[:, k, :],
                                start=(k == 0),
                                stop=False,
                            )
                        nc.tensor.matmul(
                            p[:, :CW], ones_r[:], bv[:], start=False, stop=True
                        )
                        nc.scalar.copy(v1[:, zc, :, 0:D], p[:, :CW])

                    return go

                for m in range(2):
                    thunks.append(qk("q", m))
                    thunks.append(qk("k", m))
                for zz in range(4):
                    thunks.append(vproj(zz))
                return thunks

            # ---- attention groups for chunk sc ----
            # tails[sc] collects deferred thunks: broadcast+normalize per
            # head, then the output projection for the chunk.
            tails = {sc: [] for sc in range(NSC)}

            def attn_groups(sc):
                s0 = sc * SC
                groups = []
                aos = {}
                for m in range(2):  # head pair (heads 2m, 2m+1)
                    nz = 4 * sc + 4
                    pavs = {}

                    def start_pair(m=m):
                        def go():
                            pavs[0] = pavpool.tile(
                                [D + 1, SC], f32, tag="pav", name="pav0"
                            )
                            pavs[1] = pavpool.tile(
                                [D + 1, SC], f32, tag="pav", name="pav1"
                            )

                        return go

                    def zgroup(zc, m=m, nz=nz):
                        def go():
                            diag = zc >= 4 * sc
                            j0 = 128 * (zc - 4 * sc) if diag else 0
                            ps = ps2pool.tile([128, 2, SC], f32, tag="ps2")
                            for hh in range(2):
                                po = 64 * hh
                                nc.tensor.matmul(
                                    ps[:, hh, j0:],
                                    kT[po : po + D, m, 128 * zc : 128 * (zc + 1)],
                                    qT[po : po + D, m, s0 + j0 : s0 + SC],
                                    start=True,
                                    stop=not diag,
                                )
                            if diag:
                                for hh in range(2):
                                    nc.tensor.matmul(
                                        ps[:, hh, j0 : j0 + 128],
                                        mskA[:],
                                        idn[:],
                                        start=False,
                                        stop=True,
                                    )
                            p2 = p2pool.tile([128, 2, SC], bf16, tag="p2")
                            nc.scalar.activation(
                                p2[:, :, j0:],
                                ps[:, :, j0:],
                                mybir.ActivationFunctionType.Exp,
                            )
                            for hh in range(2):
                                h = 2 * m + hh
                                nc.tensor.matmul(
                                    pavs[hh][:, j0:],
                                    v1[:, zc, h, :],
                                    p2[:, hh, j0:],
                                    start=(zc == 0),
                                    stop=(zc == nz - 1),
                                )

                        return go

                    def end_pair(m=m):
                        def go():
                            denT = denT0 if m == 0 else denT1
                            # cast h first (frees the pav bank for the next
                            # pair ASAP), its den row right after, then the
                            # reciprocal once both rows are in.
                            for hh in range(2):
                                g = 2 * m + hh
                                ao = aopool.tile([D + 1, SC], f32r, tag="aoTu")
                                nc.vector.tensor_copy(ao[:], pavs[hh][:])
                                nc.vector.tensor_copy(
                                    denT[32 * hh : 32 * hh + 1, :],
                                    ao[D : D + 1, :],
                                )
                                aos[g] = ao
                            rci = rcpool.tile([D, SC], f32, tag="rci")
                            nc.vector.reciprocal(rci[:], denT[0:D, :])
                            recT = rcpool.tile([D, SC], f32r, tag="recT", bufs=4)
                            nc.vector.tensor_scalar_min(recT[:], rci[:], 1.0e30)
                            for hh in range(2):
                                tails[sc].append(
                                    norm_thunk(m, hh, aos[2 * m + hh], recT)
                                )

                        return go

                    def norm_thunk(m, hh, ao, recT):
                        def go():
                            po = 64 * hh
                            pbd = mpool.tile([128, SC], f32, tag="misc")
                            nc.tensor.matmul(
                                pbd[0:D, :],
                                ind4[:, hh * D : (hh + 1) * D],
                                recT[:],
                                start=True,
                                stop=True,
                            )
                            nc.vector.tensor_tensor(
                                out=aoT[po : po + D, m, s0 : s0 + SC],
                                in0=ao[0:D, :],
                                in1=pbd[0:D, :],
                                op=mybir.AluOpType.mult,
                            )

                        return go

                    groups.append(start_pair())
                    gs = [zgroup(zc) for zc in range(nz)]
                    if m == 1 and sc >= 2:
                        # flush pair0's deferred normalize (bcast+mult) into
                        # pair1's attention stream, well after pair0's recip
                        # chain has drained on DVE.
                        def flush():
                            todo = tails[sc][:]
                            tails[sc].clear()
                            for t in todo:
                                t()

                        gs = gs[:7] + [flush] + gs[7:]
                    groups.extend(gs)
                    groups.append(end_pair())
                return groups

            # ---- output projection thunks for chunk sc ----
            def p3_thunks(sc):
                thunks = []
                for sb in range(4):
                    def go(sb=sb):
                        s0b = sc * SC + sb * 128
                        ost = opool.tile([128, 2, SC], f32, tag="ost")
                        po0 = mpool.tile([128, SC], f32, tag="misc")
                        po1 = mpool.tile([128, SC], f32, tag="misc")
                        for m in range(2):
                            nc.tensor.matmul(
                                po0[:],
                                aoT[:, m, s0b : s0b + 128],
                                wo[:, m, 0:SC],
                                start=(m == 0),
                                stop=(m == 1),
                            )
                            nc.tensor.matmul(
                                po1[:],
                                aoT[:, m, s0b : s0b + 128],
                                wo[:, m, SC : 2 * SC],
                                start=(m == 0),
                                stop=(m == 1),
                            )
                        nc.vector.tensor_copy(ost[:, 0, :], po0[:])
                        nc.sync.dma_start(
                            out=out_d[s0b : s0b + 128, 0:SC], in_=ost[:, 0, :]
                        )
                        nc.scalar.copy(ost[:, 1, :], po1[:])
                        nc.sync.dma_start(
                            out=out_d[s0b : s0b + 128, SC:], in_=ost[:, 1, :]
                        )

                    thunks.append(go)
                return thunks

            # ---- emission schedule ----
            # proj_thunks order: [q(m0), k(m0), q(m1), k(m1), v0..v3].
            # For the last chunk, K and V weave into attn(3) itself so the
            # tail attention stretch (which has no other projection work
            # left) keeps enough non-exp-gated PE work to hold HAM warm.
            dma_chunk(0)()
            small_consts()
            wkv_dmas()
            p0 = proj_thunks(0)
            for t in p0[:3]:
                t()
            dma_chunk(1)()
            late_consts()
            for t in p0[3:]:
                t()
            carry = []
            for sc in range(1, NSC + 1):
                early, late = [], []
                early.extend(carry)
                carry = []
                if sc < NSC:
                    if sc + 1 < NSC:
                        early.append(dma_chunk(sc + 1))
                    pt = proj_thunks(sc)
                    if sc == NSC - 1:
                        early.extend([pt[0], pt[2]])
                        carry = [pt[1], pt[3]] + pt[4:]
                    else:
                        early.extend(pt)
                if sc >= 2:
                    late.extend(tails[sc - 2])
                    late.extend(p3_thunks(sc - 2))
                _weave(attn_groups(sc - 1), early, late)
            for t in tails[NSC - 1]:
                t()
            for t in p3_thunks(NSC - 1):
                t()

    _split_excess_waits(nc)
    return nc


_CACHE = {}


def _get_nc():
    if "nc" not in _CACHE:
        _CACHE["nc"] = _build()
    return _CACHE["nc"]


def _ensure_ntff_hook():
    """The agent image's antenv lacks axon_hooks, so run_bass_kernel_spmd's
    trace path can't import it. Synthesize the module and install the
    ctypes NTFF hook from trn_agent_boot (same thing boot() would do)."""
    import sys
    import types

    if "antenv.axon_hooks" not in sys.modules:
        mod = types.ModuleType("antenv.axon_hooks")
        holder = [None]
        mod.set_axon_ntff_profile_hook = lambda h: holder.__setitem__(0, h)
        mod.get_axon_ntff_profile_hook = lambda: holder[0]
        sys.modules["antenv.axon_hooks"] = mod
        import antenv

        antenv.axon_hooks = mod
    import antenv.axon_hooks as ah

    if ah.get_axon_ntff_profile_hook() is None:
        try:
            from trn_agent_boot.trn_boot import _ntff_profile_via_ctypes

            ah.set_axon_ntff_profile_hook(
                _ntff_profile_via_ctypes("/opt/axon/libaxon_pjrt.so")
            )
        except Exception:
            pass


def _host_consts():
    # mskA[p, i] = -MASK_VAL if i >= p else 0, so that (mskA.T @ I)[i, j]
    # = mskA[j, i] = -MASK_VAL * (i >= j)  (the causal triangle).
    p = np.arange(128)[:, None]
    i = np.arange(128)[None, :]
    mskA = np.where(i >= p, -np.float32(MASK_VAL), np.float32(0.0)).astype(
        np.float32
    )
    idn = np.eye(128, dtype=np.float32)
    ind4 = np.zeros((64, 2 * D), np.float32)
    for hh in range(2):
        ind4[32 * hh, hh * D : (hh + 1) * D] = 1.0
    return _round_f32r(mskA), _round_f32r(idn), _round_f32r(ind4)


def kernel(attend_from, attend_to, w_q, b_q, w_kv, b_kv, w_out, b_out, _trace=False):
    attend_from = np.asarray(attend_from, dtype=np.float32)
    attend_to = np.asarray(attend_to, dtype=np.float32)
    w_q = np.asarray(w_q, dtype=np.float32)
    b_q = np.asarray(b_q, dtype=np.float32)
    w_kv = np.asarray(w_kv, dtype=np.float32)
    b_kv = np.asarray(b_kv, dtype=np.float32)
    w_out = np.asarray(w_out, dtype=np.float32)
    b_out = np.asarray(b_out, dtype=np.float32)

    mskA, idn, ind4 = _host_consts()
    xT = [_round_f32r(attend_from[b].T) for b in range(B)]
    xTt = [_round_f32r(attend_to[b].T) for b in range(B)]

    in_maps = []
    for c in range(NCORES):
        b, hg = divmod(c, HG)
        cols = slice(hg * CW, (hg + 1) * CW)
        in_maps.append(
            {
                "xf": xT[b],
                "xt": xTt[b],
                "wq": _round_f32r(w_q[:, cols]),
                "wk": _round_f32r(w_kv[:, cols]),
                "wv": _round_f32r(w_kv[:, F:][:, cols]),
                "wo": _round_f32r(w_out[cols, :]),
                "bq": np.ascontiguousarray(b_q[cols].reshape(CW, 1)),
                "bk": np.ascontiguousarray(b_kv[cols].reshape(CW, 1)),
                "bv": _round_f32r(b_kv[F:][cols].reshape(1, CW)),
                "mskA": mskA,
                "idn": idn,
                "ind4": ind4,
                "out": np.zeros((S, F), np.float32),
            }
        )

    nc = _get_nc()
    if _trace:
        _ensure_ntff_hook()
    res = run_bass_kernel_spmd(nc, in_maps, list(range(NCORES)), trace=_trace)

    out = np.zeros((B, S, F), np.float64)
    for c in range(NCORES):
        b = c // HG
        out[b] += res.results[c]["out"].astype(np.float64)
    out += b_out.astype(np.float64)[None, None, :]

    # Row 0 of the reference is fully masked -> softmax is exactly uniform
    # over all Z positions; compute it directly on the host.
    w_v = w_kv[:, F:].astype(np.float64)
    for b in range(B):
        val_mean = attend_to[b].astype(np.float64).mean(axis=0) @ w_v + b_kv[
            F:
        ].astype(np.float64)
        out[b, 0, :] = val_mean @ w_out.astype(np.float64) + b_out.astype(np.float64)

    if _trace:
        kernel._last_result = res
    return out.astype(np.float32)
